# revision 7
# baseline (speedup 1.0000x reference)
"""Trainium2 Bass kernel for nn_BiRNNImputerModel (bidirectional GRU imputer).

Strategy:
  - 8 cores: cores 0-3 run the forward GRU, cores 4-7 the backward GRU
    (backward = same program on time-reversed inputs).
  - Within a direction, data-parallel over batch: 128 / 4 = 32 per core.
  - Everything on-chip lives in "transposed" layout [feature/H, batch] so the
    recurrent matmuls need no per-step transposes:
        gates^T[3H, B] = W^T-stationary @ activations-moving.
  - Weights & moving operands in bf16 (fp32 PSUM accumulation), gate math in
    fp32 from PSUM.
  - Hidden-state history is kept in SBUF ([128, 4*512*32] bf16) and consumed
    directly by the final readout GEMM (partial product with this direction's
    half of Wout). The cross-direction sum + bout + layout fixes happen on the
    host (cheap numpy), so there is no cross-core communication at all.
"""

import os
import sys

for _p in ("/opt/trn_rl_repo", "/root/.axon_site/_ro/trn_rl_repo"):
    if os.path.isdir(_p) and _p not in sys.path:
        sys.path.insert(0, _p)

import numpy as np
import ml_dtypes

import concourse.bass as bass
import concourse.tile as tile
from concourse import mybir
from concourse.bass_utils import run_bass_kernel_spmd

BF16 = ml_dtypes.bfloat16

B, S, N, C = 128, 512, 64, 1
F = N * C          # 64
H = 512
NB = 32            # batch per core (128 / 4)
NFOLD = 4          # H / 128
AF = mybir.ActivationFunctionType
ALU = mybir.AluOpType


def _legalize_multiwait(nc, max_waits=1):
    """walrus in this image only encodes one sync-wait on Drain (CTRL)
    instructions; hoist extra waits onto preceding NoOps."""
    n_fix = 0
    for f in nc.m.functions:
        for blk in f.blocks:
            new = []
            for ins in blk.instructions:
                si = getattr(ins, "sync_info", None)
                if si is not None and si.on_wait and len(si.on_wait) > max_waits:
                    waits = list(si.on_wait)
                    si.on_wait = waits[-max_waits:]
                    for i, w in enumerate(waits[:-max_waits]):
                        new.append(
                            mybir.InstNoOp(
                                name=f"{ins.name}-waitfix-{i}",
                                engine=ins.engine,
                                sync_info=mybir.SyncInfo(on_wait=[w], on_update=[]),
                                bass_nofuse=True,
                            )
                        )
                        n_fix += 1
                new.append(ins)
            blk.instructions[:] = new
    return n_fix


def build_nc(n_steps):
    """Build the per-core SPMD program. n_steps = S - 1 recurrent steps."""
    nc = bass.Bass()
    dt = mybir.dt

    n_tok = n_steps * NB

    xt = nc.dram_tensor("xt", [F, n_steps, NB], dt.bfloat16, kind="ExternalInput")
    mt = nc.dram_tensor("mt", [F, n_steps, NB], dt.bfloat16, kind="ExternalInput")
    wih = nc.dram_tensor("wih", [2 * F, 3 * H], dt.bfloat16, kind="ExternalInput")
    whh = nc.dram_tensor("whh", [128, NFOLD * 3 * H], dt.bfloat16, kind="ExternalInput")
    wro = nc.dram_tensor("wro", [128, NFOLD * F], dt.bfloat16, kind="ExternalInput")
    wout = nc.dram_tensor("wout", [128, NFOLD * F], dt.bfloat16, kind="ExternalInput")
    brz = nc.dram_tensor("brz", [128, 8], dt.float32, kind="ExternalInput")
    bin_ = nc.dram_tensor("bin", [128, NFOLD], dt.float32, kind="ExternalInput")
    bhn = nc.dram_tensor("bhn", [128, NFOLD], dt.float32, kind="ExternalInput")
    bro = nc.dram_tensor("bro", [F, 1], dt.float32, kind="ExternalInput")

    xh_out = nc.dram_tensor("xh", [F, n_steps, NB], dt.float32, kind="ExternalOutput")
    pp_out = nc.dram_tensor("pp", [F, n_tok], dt.float32, kind="ExternalOutput")

    with tile.TileContext(nc) as tc:
        with (
            tc.tile_pool(name="singles", bufs=1) as singles,
            tc.tile_pool(name="hist", bufs=1) as hist,
            tc.tile_pool(name="xin", bufs=4) as xinp,
            tc.tile_pool(name="xtp", bufs=4) as xtp,
            tc.tile_pool(name="work", bufs=3) as work,
            tc.tile_pool(name="ps", bufs=1, space="PSUM") as psp,
            tc.tile_pool(name="psro", bufs=2, space="PSUM") as psro,
            tc.tile_pool(name="psf", bufs=2, space="PSUM") as psf,
            tc.tile_pool(name="outs", bufs=3) as outs,
        ):
            # --- load weights / biases (once) ---
            wih_sb = singles.tile([2 * F, 3 * H], dt.bfloat16)
            nc.sync.dma_start(out=wih_sb, in_=wih[:])
            whh_sb = singles.tile([128, NFOLD * 3 * H], dt.bfloat16)
            nc.sync.dma_start(out=whh_sb, in_=whh[:])
            wro_sb = singles.tile([128, NFOLD * F], dt.bfloat16)
            nc.sync.dma_start(out=wro_sb, in_=wro[:])
            wout_sb = singles.tile([128, NFOLD * F], dt.bfloat16)
            nc.sync.dma_start(out=wout_sb, in_=wout[:])
            brz_sb = singles.tile([128, 8], dt.float32)
            nc.sync.dma_start(out=brz_sb, in_=brz[:])
            bin_sb = singles.tile([128, NFOLD], dt.float32)
            nc.sync.dma_start(out=bin_sb, in_=bin_[:])
            bhn_sb = singles.tile([128, NFOLD], dt.float32)
            nc.sync.dma_start(out=bhn_sb, in_=bhn[:])
            bro_sb = singles.tile([F, 1], dt.float32)
            nc.sync.dma_start(out=bro_sb, in_=bro[:])

            # hidden-state history, fold-major: [128, c, t, b]; t=0 is h0=0
            h_hist = hist.tile([128, NFOLD, (n_steps + 1) * NB], dt.bfloat16)
            nc.vector.memset(h_hist[:, :, 0:NB], 0.0)

            # x_in for step 1: rows 0:64 <- bro (xhat_0), rows 64:128 <- m_0
            x_in = xinp.tile([2 * F, NB], dt.bfloat16)
            nc.sync.dma_start(out=x_in[F : 2 * F, :], in_=mt[:, 0, :])
            nc.vector.memset(x_in[0:F, :], 0.0)
            nc.scalar.activation(
                out=x_in[0:F, :], in_=x_in[0:F, :], func=AF.Identity,
                bias=bro_sb[:, 0:1], scale=1.0,
            )
            xt_t = xtp.tile([F, NB], dt.bfloat16, tag="xt_t")
            nc.sync.dma_start(out=xt_t, in_=xt[:, 0, :])
            mk_t = xtp.tile([F, NB], dt.bfloat16, tag="mk_t")
            nc.sync.dma_start(out=mk_t, in_=mt[:, 0, :])
            nc.vector.copy_predicated(
                x_in[0:F, :], mk_t.bitcast(mybir.dt.uint16), xt_t
            )

            for t in range(1, n_steps + 1):
                hprev = lambda c: h_hist[:, c, (t - 1) * NB : t * NB]
                ps_ro = psro.tile([F, NB], dt.float32, tag="ps_ro")
                r_t = work.tile([128, NFOLD * NB], dt.bfloat16, tag="r_t")
                z_t = work.tile([128, NFOLD * NB], dt.bfloat16, tag="z_t")
                n_t = work.tile([128, NFOLD * NB], dt.bfloat16, tag="n_t")

                for c in range(NFOLD):
                    cs = slice(c * NB, (c + 1) * NB)
                    ps = psp.tile([128, 4 * NB], dt.float32, tag=f"ps{c}")
                    # regions: [0:32]=r, [32:64]=z, [64:96]=gi_n, [96:128]=gh_n
                    for j, g in enumerate((c, 4 + c, 8 + c)):
                        wsl = slice(128 * g, 128 * (g + 1))
                        if j < 2:
                            dst = ps[:, j * NB : (j + 1) * NB]
                            nc.tensor.matmul(dst, wih_sb[:, wsl], x_in, start=True, stop=False)
                            for c2 in range(NFOLD):
                                nc.tensor.matmul(
                                    dst, whh_sb[:, c2 * 3 * H + 128 * g : c2 * 3 * H + 128 * (g + 1)],
                                    hprev(c2), start=False, stop=(c2 == NFOLD - 1),
                                )
                        else:
                            nc.tensor.matmul(ps[:, 2 * NB : 3 * NB], wih_sb[:, wsl], x_in,
                                             start=True, stop=True)
                            dst = ps[:, 3 * NB : 4 * NB]
                            for c2 in range(NFOLD):
                                nc.tensor.matmul(
                                    dst, whh_sb[:, c2 * 3 * H + 128 * g : c2 * 3 * H + 128 * (g + 1)],
                                    hprev(c2), start=(c2 == 0), stop=(c2 == NFOLD - 1),
                                )

                    # gate math for fold c
                    nc.scalar.activation(out=r_t[:, cs], in_=ps[:, 0:NB], func=AF.Sigmoid,
                                         bias=brz_sb[:, c : c + 1])
                    nc.scalar.activation(out=z_t[:, cs], in_=ps[:, NB : 2 * NB], func=AF.Sigmoid,
                                         bias=brz_sb[:, 4 + c : 5 + c])
                    tmp_c = work.tile([128, NB], dt.float32, tag="tmp_c")
                    nc.vector.scalar_tensor_tensor(
                        out=tmp_c, in0=ps[:, 3 * NB : 4 * NB], scalar=bhn_sb[:, c : c + 1],
                        in1=r_t[:, cs], op0=ALU.add, op1=ALU.mult,
                    )
                    nin_c = work.tile([128, NB], dt.float32, tag="nin_c")
                    nc.vector.tensor_tensor(nin_c, tmp_c, ps[:, 2 * NB : 3 * NB], ALU.add)
                    nc.scalar.activation(out=n_t[:, cs], in_=nin_c, func=AF.Tanh,
                                         bias=bin_sb[:, c : c + 1])
                    hmn_c = work.tile([128, NB], dt.float32, tag="hmn_c")
                    nc.vector.tensor_tensor(hmn_c, hprev(c), n_t[:, cs], ALU.subtract)
                    zh_c = work.tile([128, NB], dt.float32, tag="zh_c")
                    nc.vector.tensor_tensor(zh_c, hmn_c, z_t[:, cs], ALU.mult)
                    hnew_c = h_hist[:, c, t * NB : (t + 1) * NB]
                    nc.vector.tensor_tensor(hnew_c, zh_c, n_t[:, cs], ALU.add)

                    # readout accumulation for this fold
                    nc.tensor.matmul(ps_ro, wro_sb[:, c * F : (c + 1) * F], hnew_c,
                                     start=(c == 0), stop=(c == NFOLD - 1))

                # xhat_t (f32 for output)
                xhat_f = outs.tile([F, NB], dt.float32, tag="xhat_f")
                nc.scalar.activation(out=xhat_f, in_=ps_ro, func=AF.Identity,
                                     bias=bro_sb[:, 0:1])
                nc.sync.dma_start(out=xh_out[:, t - 1, :], in_=xhat_f)

                if t < n_steps:
                    # build x_in for step t+1
                    x_in = xinp.tile([2 * F, NB], dt.bfloat16)
                    nc.sync.dma_start(out=x_in[F : 2 * F, :], in_=mt[:, t, :])
                    nc.scalar.activation(out=x_in[0:F, :], in_=ps_ro, func=AF.Identity,
                                         bias=bro_sb[:, 0:1])
                    xt_t = xtp.tile([F, NB], dt.bfloat16, tag="xt_t")
                    nc.sync.dma_start(out=xt_t, in_=xt[:, t, :])
                    mk_t = xtp.tile([F, NB], dt.bfloat16, tag="mk_t")
                    nc.sync.dma_start(out=mk_t, in_=mt[:, t, :])
                    nc.vector.copy_predicated(
                        x_in[0:F, :], mk_t.bitcast(mybir.dt.uint16), xt_t
                    )

            # --- final readout partial GEMM: pp = wout^T-half @ h  over all tokens
            TOK_TILE = 512
            for t0 in range(0, n_tok, TOK_TILE):
                ntk = min(TOK_TILE, n_tok - t0)
                ps_p = psf.tile([F, TOK_TILE], dt.float32, tag="ps_p")
                for c in range(NFOLD):
                    nc.tensor.matmul(
                        ps_p[:, 0:ntk], wout_sb[:, c * F : (c + 1) * F],
                        h_hist[:, c, NB + t0 : NB + t0 + ntk],
                        start=(c == 0), stop=(c == NFOLD - 1),
                    )
                p_sb = outs.tile([F, TOK_TILE], dt.float32, tag="p_sb")
                nc.scalar.activation(out=p_sb[:, 0:ntk], in_=ps_p[:, 0:ntk], func=AF.Copy)
                nc.sync.dma_start(out=pp_out[:, t0 : t0 + ntk], in_=p_sb[:, 0:ntk])

    _legalize_multiwait(nc)
    return nc


_NC_CACHE = {}


def _get_nc(n_steps):
    if n_steps not in _NC_CACHE:
        _NC_CACHE[n_steps] = build_nc(n_steps)
    return _NC_CACHE[n_steps]


def _prep_core_inputs(x2d, m2d, Wih, Whh, bih, bhh, Wro, bro, Wout_half, n_steps):
    """Per-core input map. x2d/m2d: [NB, S_loc, F] float32/bool already
    direction-ordered (time-reversed for backward cores)."""
    xt = np.ascontiguousarray(x2d[:, :n_steps].transpose(2, 1, 0)).astype(BF16)
    mt = np.ascontiguousarray(m2d[:, :n_steps].transpose(2, 1, 0).astype(np.float32)).astype(BF16)
    wih_t = np.ascontiguousarray(Wih.T).astype(BF16)                      # [128, 3H]
    whh_t = np.ascontiguousarray(
        Whh.T.reshape(NFOLD, 128, 3 * H).transpose(1, 0, 2).reshape(128, NFOLD * 3 * H)
    ).astype(BF16)
    wro_t = np.ascontiguousarray(
        Wro.T.reshape(NFOLD, 128, F).transpose(1, 0, 2).reshape(128, NFOLD * F)
    ).astype(BF16)
    wout_t = np.ascontiguousarray(
        Wout_half.T.reshape(NFOLD, 128, F).transpose(1, 0, 2).reshape(128, NFOLD * F)
    ).astype(BF16)
    bsum = bih + bhh
    brz = np.stack([bsum[128 * g : 128 * (g + 1)] for g in range(8)], axis=1).astype(np.float32)
    bin_ = np.stack([bih[1024 + 128 * c : 1024 + 128 * (c + 1)] for c in range(NFOLD)], axis=1).astype(np.float32)
    bhn = np.stack([bhh[1024 + 128 * c : 1024 + 128 * (c + 1)] for c in range(NFOLD)], axis=1).astype(np.float32)
    return {
        "xt": xt, "mt": mt, "wih": wih_t, "whh": whh_t, "wro": wro_t,
        "wout": wout_t, "brz": brz, "bin": bin_, "bhn": bhn,
        "bro": bro.reshape(F, 1).astype(np.float32),
    }


def run_device(inputs, s_len=S, trace=False):
    """Run the 8-core SPMD kernel. Returns (results_list, bass_results)."""
    n_steps = s_len - 1
    nc = _get_nc(n_steps)

    x2d = np.asarray(inputs["x"], np.float32).reshape(B, S, F)[:, :s_len]
    m2d = np.asarray(inputs["mask"]).reshape(B, S, F)[:, :s_len]

    in_maps = []
    for core in range(8):
        g = core % 4
        bsl = slice(NB * g, NB * (g + 1))
        if core < 4:
            xs, ms = x2d[bsl], m2d[bsl]
            im = _prep_core_inputs(
                xs, ms, inputs["Wih_f"], inputs["Whh_f"], inputs["bih_f"],
                inputs["bhh_f"], inputs["Wro_f"], inputs["bro_f"],
                np.asarray(inputs["Wout"])[:, :H], n_steps,
            )
        else:
            xs, ms = x2d[bsl, ::-1], m2d[bsl, ::-1]
            im = _prep_core_inputs(
                xs, ms, inputs["Wih_b"], inputs["Whh_b"], inputs["bih_b"],
                inputs["bhh_b"], inputs["Wro_b"], inputs["bro_b"],
                np.asarray(inputs["Wout"])[:, H:], n_steps,
            )
        in_maps.append(im)

    res = run_bass_kernel_spmd(nc, in_maps, core_ids=list(range(8)), trace=trace)
    return res


def assemble(inputs, res, s_len=S):
    """Host-side gather: combine per-core outputs into full reference outputs."""
    n_steps = s_len - 1
    bro_f = np.asarray(inputs["bro_f"], np.float32)
    bro_b = np.asarray(inputs["bro_b"], np.float32)
    bout = np.asarray(inputs["bout"], np.float32)

    xh_f = np.empty((B, s_len, F), np.float32)
    xh_b = np.empty((B, s_len, F), np.float32)
    x_hat = np.empty((B, s_len, F), np.float32)

    for g in range(4):
        bsl = slice(NB * g, NB * (g + 1))
        rf, rb = res.results[g], res.results[g + 4]
        # xh: [F, n_steps, NB] -> [NB, t, F]
        xh_f[bsl, 1:] = rf["xh"].transpose(2, 1, 0)
        xh_f[bsl, 0] = bro_f
        xh_b[bsl, :n_steps] = rb["xh"].transpose(2, 1, 0)[:, ::-1]
        xh_b[bsl, n_steps] = bro_b
        pf = rf["pp"].reshape(F, n_steps, NB).transpose(2, 1, 0)
        pb = rb["pp"].reshape(F, n_steps, NB).transpose(2, 1, 0)[:, ::-1]
        x_hat[bsl, 1:] = pf
        x_hat[bsl, 0] = 0.0
        x_hat[bsl, :n_steps] += pb
        x_hat[bsl] += bout

    return (
        x_hat.reshape(B, s_len, N, C),
        xh_f.reshape(B, s_len, N, C),
        xh_b.reshape(B, s_len, N, C),
    )


def kernel(**inputs):
    res = run_device(inputs, s_len=S)
    return assemble(inputs, res, s_len=S)
